# revision 16
# baseline (speedup 1.0000x reference)
"""Trainium2 Bass kernel for DrafterAttention (decode attention, B=8 H=16 D=128 S=4096 HID=2048).

Strategy (tensor-parallel over heads, 8 cores x 2 heads):
  - Host: shard Wq columns / Wo rows / kv on the head axis; pre-transpose
    kv_k -> (B,HC,D,S) and pre-tile kv_v -> (B,HC,128,NCH*128) so every
    device-side DMA moves contiguous per-partition lines and every matmul
    consumes natural SBUF layouts; pre-transpose x -> (HID,B).
  - Device (per core): qT = Wq_shard^T @ x^T on the PE; RMS-norm + RoPE in a
    (d-on-partition, batch-on-free) layout; per (b,h): 32 matmuls
    kT_chunk^T @ q_col -> scores (128s x 32chunk) in one PSUM accumulation
    group; exp via ACT with fused row-sum (no max subtraction: logits are O(1)
    by construction); partition-sum broadcast via ones-matmul; 32 accumulating
    matmuls vT_chunk^T @ prob_col -> attention head column; o_proj partial
    (8 x 2048) per core.
  - Host: sum the 8 partial outputs (the all-reduce).

KV/attention matmuls run in KV_DTYPE ("bf16" halves HBM traffic and runs the
PE weight path at FWL speed; "f32" is the exact fallback). The q/o projections
always run in f32.
"""
import numpy as np

B, H, D, S, HID = 8, 16, 128, 4096, 2048
NCORES = 8
HC = H // NCORES          # 2 heads per core
NCH = S // 128            # 32 s-chunks
SCALE = 1.0 / np.sqrt(D)
EPS = 1e-6

KV_DTYPE = "bf16"         # "bf16" | "f32"

_CACHE = {}


def _split_sync_waits(nc, max_waits=1):
    """This walrus build caps per-instruction sem waits; hoist any excess
    onto NoOp instructions inserted just before, on the same engine."""
    from concourse import mybir
    import bass_rust

    n = 0
    for fn in nc.m.functions:
        for blk in fn.blocks:
            new_list = []
            changed = False
            for inst in blk.instructions:
                si = inst.sync_info
                waits = list(si.on_wait) if (si and si.on_wait) else []
                if len(waits) > max_waits:
                    extra, keep = waits[:-max_waits], waits[-max_waits:]
                    for i in range(0, len(extra), max_waits):
                        n += 1
                        nop = bass_rust.InstNoOp(
                            name=f"I-waitsplit-{n}", ins=[], outs=[])
                        nop.engine = inst.engine
                        nop.sync_info = mybir.SyncInfo(
                            on_wait=extra[i:i + max_waits], on_update=[])
                        new_list.append(nop)
                    si.on_wait = keep
                    changed = True
                new_list.append(inst)
            if changed:
                blk.instructions[:] = new_list
    return n


def _build_nc(kv_dtype=KV_DTYPE):
    from contextlib import ExitStack
    import concourse.bass as bass
    import concourse.tile as tile
    from concourse import mybir

    f32 = mybir.dt.float32
    kv_dt = mybir.dt.bfloat16 if kv_dtype == "bf16" else f32

    nc = bass.Bass(trn_type="TRN2")

    x_in = nc.dram_tensor("x_in", [B, HID], f32, kind="ExternalInput")
    eye8 = nc.dram_tensor("eye8", [B, B], f32, kind="ExternalInput")
    wq = nc.dram_tensor("wq", [HID, HC * D], f32, kind="ExternalInput")
    wo = nc.dram_tensor("wo", [HC * D, HID], f32, kind="ExternalInput")
    kT = nc.dram_tensor("kT", [B, HC, D, S], kv_dt, kind="ExternalInput")
    # vT is pre-tiled on host: vT[b,h][p, c*128+d] = kv_v[b,h][d, c*128+p]
    vT = nc.dram_tensor("vT", [B, HC, 128, S], kv_dt, kind="ExternalInput")
    # csg: col0 = [cos;sin] stacked (128), col1 = gamma*SCALE (128)
    csg = nc.dram_tensor("csg", [D, 2], f32, kind="ExternalInput")
    mk = nc.dram_tensor("mk", [B, 128, NCH], f32, kind="ExternalInput")
    out = nc.dram_tensor("out", [B, HID], f32, kind="ExternalOutput")

    with ExitStack() as ctx:
        tc = ctx.enter_context(tile.TileContext(nc))

        consts = ctx.enter_context(tc.tile_pool(name="consts", bufs=1))
        qpool = ctx.enter_context(tc.tile_pool(name="qpool", bufs=1))
        kpool = ctx.enter_context(tc.tile_pool(name="kpool", bufs=4))
        vpool = ctx.enter_context(tc.tile_pool(name="vpool", bufs=4))
        spool = ctx.enter_context(tc.tile_pool(name="spool", bufs=2))

        # ---- constants / small inputs ----
        ones_sb = consts.tile([128, 128], f32)
        nc.gpsimd.memset(ones_sb[:], 1.0)
        eps_sb = consts.tile([128, 1], f32)
        nc.gpsimd.memset(eps_sb[:], EPS)
        csg_sb = consts.tile([D, 2], f32)
        nc.sync.dma_start(csg_sb[:], csg[:])
        x_sb = consts.tile([B, HID], f32)
        nc.sync.dma_start(x_sb[:], x_in[:])
        eye_sb = consts.tile([B, B], f32)
        nc.sync.dma_start(eye_sb[:], eye8[:])
        wq_sb = consts.tile([128, HID // 128, HC * D], f32)
        nc.sync.dma_start(wq_sb[:], wq[:].rearrange("(i p) j -> p i j", p=128))
        wo_sb = consts.tile([128, HC, HID], f32)
        nc.scalar.dma_start(wo_sb[:], wo[:].rearrange("(h p) n -> p h n", p=128))
        # transpose x on the PE: 16 tiles (B,128) -> (128,B)
        xT_sb = consts.tile([128, HID // 128, B], f32)
        with tc.tile_pool(name="psX", bufs=4, space="PSUM") as psx:
            for i in range(HID // 128):
                xt_ps = psx.tile([128, B], f32, name="xtps")
                nc.tensor.transpose(xt_ps[:], x_sb[:, i * 128:(i + 1) * 128],
                                    eye_sb[:])
                nc.scalar.copy(xT_sb[:, i, :], xt_ps[:])
        mask_tiles = []
        for b in range(B):
            mt = consts.tile([128, NCH], f32, name=f"mask{b}")
            nc.gpsimd.dma_start(mt[:], mk[b])
            mask_tiles.append(mt)

        # ---- q projection: qT_h = (Wq_h)^T @ x^T  -> (128d, B) per head ----
        q_heads = []
        with tc.tile_pool(name="psQ", bufs=1, space="PSUM") as psq:
            for h in range(HC):
                q_ps = psq.tile([128, B], f32, name=f"qps{h}")
                for i in range(HID // 128):
                    nc.tensor.matmul(
                        q_ps[:],
                        wq_sb[:, i, h * D:(h + 1) * D],
                        xT_sb[:, i, :],
                        start=(i == 0), stop=(i == HID // 128 - 1),
                    )
                # RMS norm (over the partition axis d) via ones-matmul
                sq = qpool.tile([128, B], f32, name=f"sq{h}")
                nc.scalar.square(sq[:], q_ps[:])
                ssq_ps = psq.tile([128, B], f32, name=f"ssq{h}")
                nc.tensor.matmul(ssq_ps[:], ones_sb[:], sq[:], start=True, stop=True)
                rms = qpool.tile([128, B], f32, name=f"rms{h}")
                nc.scalar.activation(rms[:], ssq_ps[:],
                                     mybir.ActivationFunctionType.Sqrt,
                                     bias=eps_sb[:], scale=1.0 / D)
                nc.vector.reciprocal(rms[:], rms[:])
                qn = qpool.tile([128, B], f32, name=f"qn{h}")
                nc.vector.tensor_mul(qn[:], q_ps[:], rms[:])
                # gamma * SCALE (per-partition scalar)
                nc.vector.tensor_scalar_mul(qn[:], qn[:], csg_sb[:, 1:2])
                # RoPE on partition halves: cos/sin stacked in csg col 0
                qr = qpool.tile([128, B], kv_dt, name=f"qr{h}")
                t1 = qpool.tile([64, B], f32, name=f"t1{h}")
                t2 = qpool.tile([64, B], f32, name=f"t2{h}")
                cos_ap = csg_sb[0:64, 0:1]
                sin_ap = csg_sb[64:128, 0:1]
                q1 = qn[0:64, :]
                q2 = qn[64:128, :]
                nc.vector.tensor_scalar_mul(t1[:], q1, cos_ap)
                nc.vector.tensor_scalar_mul(t2[:], q2, sin_ap)
                nc.vector.tensor_sub(qr[0:64, :], t1[:], t2[:])
                nc.vector.tensor_scalar_mul(t1[:], q2, cos_ap)
                nc.vector.tensor_scalar_mul(t2[:], q1, sin_ap)
                nc.vector.tensor_add(qr[64:128, :], t1[:], t2[:])
                q_heads.append(qr)

        # attention output columns, (128d, B) per head
        at_tiles = [qpool.tile([128, B], f32, name=f"at{h}") for h in range(HC)]
        o_sb = qpool.tile([B, HID], f32, name="osb")

        # ---- main streamed attention loop (h-major; AV pipelined 1 unit
        # behind scores so the PE never waits on the ACT/DVE softmax chain) --
        units = [(h, b) for h in range(HC) for b in range(B)]
        # prefetch the LAST unit's k/v first so the tail isn't DMA-bound
        lastpool = ctx.enter_context(tc.tile_pool(name="lastkv", bufs=1))
        hl, bl = units[-1]
        k_last = lastpool.tile([128, S], kv_dt, name="klast")
        nc.sync.dma_start(k_last[:], kT[bl, hl])
        v_last = lastpool.tile([128, NCH, 128], kv_dt, name="vlast")
        nc.scalar.dma_start(v_last[:], vT[bl, hl])

        ps_sc = ctx.enter_context(tc.tile_pool(name="psS", bufs=2, space="PSUM"))
        ps_av = ctx.enter_context(tc.tile_pool(name="psV", bufs=2, space="PSUM"))
        ps_tot = ctx.enter_context(tc.tile_pool(name="psT", bufs=2, space="PSUM"))
        ps_o = ctx.enter_context(tc.tile_pool(name="psO", bufs=2, space="PSUM"))

        def emit_av(pend):
            v_p, probs_p, rowsum_p, h_p, b_p = pend
            av_ps = ps_av.tile([128, 1], f32, name="avps")
            for c in range(NCH):
                nc.tensor.matmul(
                    av_ps[:],
                    v_p[:, c, :],
                    probs_p[:, c:c + 1],
                    start=(c == 0), stop=(c == NCH - 1),
                )
            tot_ps = ps_tot.tile([128, 1], f32, name="totps")
            nc.tensor.matmul(tot_ps[:], ones_sb[:], rowsum_p[:],
                             start=True, stop=True)
            inv = spool.tile([128, 1], f32, name="inv")
            nc.vector.reciprocal(inv[:], tot_ps[:])
            nc.scalar.activation(at_tiles[h_p][:, b_p:b_p + 1], av_ps[:],
                                 mybir.ActivationFunctionType.Copy,
                                 scale=inv[:])

        def emit_oproj(h, first):
            # partial o_proj for one head into o_sb
            for n in range(HID // 512):
                o_ps = ps_o.tile([B, 512], f32, name="ops")
                nc.tensor.matmul(o_ps[:], at_tiles[h][:],
                                 wo_sb[:, h, n * 512:(n + 1) * 512],
                                 start=True, stop=True)
                if first:
                    nc.vector.tensor_copy(o_sb[:, n * 512:(n + 1) * 512], o_ps[:])
                else:
                    nc.vector.tensor_add(o_sb[:, n * 512:(n + 1) * 512],
                                         o_sb[:, n * 512:(n + 1) * 512], o_ps[:])

        pending = None
        for u, (h, b) in enumerate(units):
            last = (u == len(units) - 1)
            if last:
                k_sb, v_sb = k_last, v_last
            else:
                k_sb = kpool.tile([128, S], kv_dt, name="ktile")
                nc.sync.dma_start(k_sb[:], kT[b, h])
                v_sb = vpool.tile([128, NCH, 128], kv_dt, name="vtile")
                nc.scalar.dma_start(v_sb[:], vT[b, h])

            q_col = q_heads[h][:, b:b + 1]
            sc_ps = ps_sc.tile([128, NCH], f32, name="scps")
            for c in range(NCH):
                nc.tensor.matmul(
                    sc_ps[:, c:c + 1],
                    k_sb[:, c * 128:(c + 1) * 128],
                    q_col,
                    start=(c == 0), stop=(c == NCH - 1),
                )
            if pending is not None:
                emit_av(pending)
            sc_sb = spool.tile([128, NCH], f32, name="scsb")
            nc.vector.tensor_add(sc_sb[:], sc_ps[:], mask_tiles[b][:])
            probs = spool.tile([128, NCH], kv_dt, name="probs")
            rowsum = spool.tile([128, 1], f32, name="rowsum")
            nc.scalar.activation(probs[:], sc_sb[:],
                                 mybir.ActivationFunctionType.Exp,
                                 accum_out=rowsum[:])
            pending = (v_sb, probs, rowsum, h, b)
            if u == B + 1:
                # head 0's attention columns are complete; project mid-loop
                emit_oproj(0, first=True)
        emit_av(pending)
        emit_oproj(1, first=False)
        nc.sync.dma_start(out[:], o_sb[:])

    _split_sync_waits(nc)
    return nc


def _get_nc():
    if "nc" not in _CACHE:
        _CACHE["nc"] = _build_nc()
    return _CACHE["nc"]


def _shard_inputs(x, kv_k, kv_v, cos, sin, mask, Wq, Wo, q_gamma, kv_dtype=KV_DTYPE):
    if kv_dtype == "bf16":
        import ml_dtypes
        kv_np = ml_dtypes.bfloat16
    else:
        kv_np = np.float32

    x = np.ascontiguousarray(np.asarray(x, np.float32).reshape(B, HID))
    eye = np.eye(B, dtype=np.float32)
    csg = np.empty((D, 2), np.float32)
    csg[:64, 0] = np.asarray(cos, np.float32).reshape(-1)
    csg[64:, 0] = np.asarray(sin, np.float32).reshape(-1)
    csg[:, 1] = np.asarray(q_gamma, np.float32).reshape(-1) * SCALE
    mk = np.ascontiguousarray(
        np.asarray(mask, np.float32).reshape(B, NCH, 128).transpose(0, 2, 1))
    kv_k = np.asarray(kv_k, np.float32)
    kv_v = np.asarray(kv_v, np.float32)
    Wq = np.asarray(Wq, np.float32)
    Wo = np.asarray(Wo, np.float32)

    in_maps = []
    for c in range(NCORES):
        hs = c * HC
        # kT[b,hl] = kv_k[b,hs+hl]^T  (D, S) contiguous
        kt = np.ascontiguousarray(
            kv_k[:, hs:hs + HC].transpose(0, 1, 3, 2).astype(kv_np))
        # vT[b,hl][p, c*128+d] = kv_v[b,hs+hl][d, c*128+p]  (pre-tiled)
        vt = np.ascontiguousarray(
            kv_v[:, hs:hs + HC].reshape(B, HC, D, NCH, 128)
            .transpose(0, 1, 4, 3, 2).reshape(B, HC, 128, S).astype(kv_np))
        in_maps.append({
            "x_in": x,
            "eye8": eye,
            "wq": np.ascontiguousarray(Wq[:, hs * D:(hs + HC) * D]),
            "wo": np.ascontiguousarray(Wo[hs * D:(hs + HC) * D, :]),
            "kT": kt,
            "vT": vt,
            "csg": csg,
            "mk": mk,
        })
    return in_maps


def kernel(x, kv_k, kv_v, cos, sin, mask, Wq, Wo, q_gamma, _trace=False):
    from concourse.bass_utils import run_bass_kernel_spmd

    nc = _get_nc()
    in_maps = _shard_inputs(x, kv_k, kv_v, cos, sin, mask, Wq, Wo, q_gamma)
    res = run_bass_kernel_spmd(nc, in_maps, list(range(NCORES)), trace=_trace)
    acc = np.zeros((B, HID), np.float64)
    for c in range(NCORES):
        acc += res.results[c]["out"].astype(np.float64)
    out = acc.astype(np.float32).reshape(B, 1, HID)
    if _trace:
        return out, res
    return out


# revision 17
# speedup vs baseline: 1.0715x; 1.0715x over previous
"""Trainium2 Bass kernel for DrafterAttention (decode attention, B=8 H=16 D=128 S=4096 HID=2048).

Strategy (tensor-parallel over heads, 8 cores x 2 heads):
  - Host: shard Wq columns / Wo rows / kv on the head axis; pre-transpose
    kv_k -> (B,HC,D,S) and pre-tile kv_v -> (B,HC,128,NCH*128) so every
    device-side DMA moves contiguous per-partition lines and every matmul
    consumes natural SBUF layouts; pre-transpose x -> (HID,B).
  - Device (per core): qT = Wq_shard^T @ x^T on the PE; RMS-norm + RoPE in a
    (d-on-partition, batch-on-free) layout; per (b,h): 32 matmuls
    kT_chunk^T @ q_col -> scores (128s x 32chunk) in one PSUM accumulation
    group; exp via ACT with fused row-sum (no max subtraction: logits are O(1)
    by construction); partition-sum broadcast via ones-matmul; 32 accumulating
    matmuls vT_chunk^T @ prob_col -> attention head column; o_proj partial
    (8 x 2048) per core.
  - Host: sum the 8 partial outputs (the all-reduce).

KV/attention matmuls run in KV_DTYPE ("bf16" halves HBM traffic and runs the
PE weight path at FWL speed; "f32" is the exact fallback). The q/o projections
always run in f32.
"""
import numpy as np

B, H, D, S, HID = 8, 16, 128, 4096, 2048
NCORES = 8
HC = H // NCORES          # 2 heads per core
NCH = S // 128            # 32 s-chunks
SCALE = 1.0 / np.sqrt(D)
EPS = 1e-6

KV_DTYPE = "bf16"         # "bf16" | "f32"

_CACHE = {}


def _split_sync_waits(nc, max_waits=1):
    """This walrus build caps per-instruction sem waits; hoist any excess
    onto NoOp instructions inserted just before, on the same engine."""
    from concourse import mybir
    import bass_rust

    n = 0
    for fn in nc.m.functions:
        for blk in fn.blocks:
            new_list = []
            changed = False
            for inst in blk.instructions:
                si = inst.sync_info
                waits = list(si.on_wait) if (si and si.on_wait) else []
                if len(waits) > max_waits:
                    extra, keep = waits[:-max_waits], waits[-max_waits:]
                    for i in range(0, len(extra), max_waits):
                        n += 1
                        nop = bass_rust.InstNoOp(
                            name=f"I-waitsplit-{n}", ins=[], outs=[])
                        nop.engine = inst.engine
                        nop.sync_info = mybir.SyncInfo(
                            on_wait=extra[i:i + max_waits], on_update=[])
                        new_list.append(nop)
                    si.on_wait = keep
                    changed = True
                new_list.append(inst)
            if changed:
                blk.instructions[:] = new_list
    return n


def _build_nc(kv_dtype=KV_DTYPE):
    from contextlib import ExitStack
    import concourse.bass as bass
    import concourse.tile as tile
    from concourse import mybir

    f32 = mybir.dt.float32
    kv_dt = mybir.dt.bfloat16 if kv_dtype == "bf16" else f32

    nc = bass.Bass(trn_type="TRN2")

    x_in = nc.dram_tensor("x_in", [B, HID], f32, kind="ExternalInput")
    eye8 = nc.dram_tensor("eye8", [B, B], f32, kind="ExternalInput")
    wq = nc.dram_tensor("wq", [HID, HC * D], kv_dt, kind="ExternalInput")
    wo = nc.dram_tensor("wo", [HC * D, HID], kv_dt, kind="ExternalInput")
    kT = nc.dram_tensor("kT", [B, HC, D, S], kv_dt, kind="ExternalInput")
    # vT is pre-tiled on host: vT[b,h][p, c*128+d] = kv_v[b,h][d, c*128+p]
    vT = nc.dram_tensor("vT", [B, HC, 128, S], kv_dt, kind="ExternalInput")
    # csg: col0 = [cos;sin] stacked (128), col1 = gamma*SCALE (128)
    csg = nc.dram_tensor("csg", [D, 2], f32, kind="ExternalInput")
    mk = nc.dram_tensor("mk", [B, 128, NCH], f32, kind="ExternalInput")
    out = nc.dram_tensor("out", [B, HID], f32, kind="ExternalOutput")

    with ExitStack() as ctx:
        tc = ctx.enter_context(tile.TileContext(nc))

        consts = ctx.enter_context(tc.tile_pool(name="consts", bufs=1))
        qpool = ctx.enter_context(tc.tile_pool(name="qpool", bufs=1))
        kpool = ctx.enter_context(tc.tile_pool(name="kpool", bufs=6))
        vpool = ctx.enter_context(tc.tile_pool(name="vpool", bufs=6))
        spool = ctx.enter_context(tc.tile_pool(name="spool", bufs=2))

        # ---- constants / small inputs ----
        ones_sb = consts.tile([128, 128], f32)
        nc.gpsimd.memset(ones_sb[:], 1.0)
        eps_sb = consts.tile([128, 1], f32)
        nc.gpsimd.memset(eps_sb[:], EPS)
        csg_sb = consts.tile([D, 2], f32)
        nc.sync.dma_start(csg_sb[:], csg[:])
        x_sb = consts.tile([B, HID], f32)
        nc.sync.dma_start(x_sb[:], x_in[:])
        eye_sb = consts.tile([B, B], f32)
        nc.sync.dma_start(eye_sb[:], eye8[:])
        wq_sb = consts.tile([128, HID // 128, HC * D], kv_dt)
        nc.sync.dma_start(wq_sb[:], wq[:].rearrange("(i p) j -> p i j", p=128))
        wo_sb = consts.tile([128, HC, HID], kv_dt)
        nc.scalar.dma_start(wo_sb[:], wo[:].rearrange("(h p) n -> p h n", p=128))
        # transpose x on the PE: 16 tiles (B,128) -> (128,B)
        xT_sb = consts.tile([128, HID // 128, B], kv_dt)
        with tc.tile_pool(name="psX", bufs=4, space="PSUM") as psx:
            for i in range(HID // 128):
                xt_ps = psx.tile([128, B], f32, name="xtps")
                nc.tensor.transpose(xt_ps[:], x_sb[:, i * 128:(i + 1) * 128],
                                    eye_sb[:])
                nc.scalar.copy(xT_sb[:, i, :], xt_ps[:])
        mask_tiles = []
        for b in range(B):
            mt = consts.tile([128, NCH], f32, name=f"mask{b}")
            nc.gpsimd.dma_start(mt[:], mk[b])
            mask_tiles.append(mt)

        # ---- q projection: qT_h = (Wq_h)^T @ x^T  -> (128d, B) per head ----
        q_heads = []
        with tc.tile_pool(name="psQ", bufs=1, space="PSUM") as psq:
            for h in range(HC):
                q_ps = psq.tile([128, B], f32, name=f"qps{h}")
                for i in range(HID // 128):
                    nc.tensor.matmul(
                        q_ps[:],
                        wq_sb[:, i, h * D:(h + 1) * D],
                        xT_sb[:, i, :],
                        start=(i == 0), stop=(i == HID // 128 - 1),
                    )
                # RMS norm (over the partition axis d) via ones-matmul
                sq = qpool.tile([128, B], f32, name=f"sq{h}")
                nc.scalar.square(sq[:], q_ps[:])
                ssq_ps = psq.tile([128, B], f32, name=f"ssq{h}")
                nc.tensor.matmul(ssq_ps[:], ones_sb[:], sq[:], start=True, stop=True)
                rms = qpool.tile([128, B], f32, name=f"rms{h}")
                nc.scalar.activation(rms[:], ssq_ps[:],
                                     mybir.ActivationFunctionType.Sqrt,
                                     bias=eps_sb[:], scale=1.0 / D)
                nc.vector.reciprocal(rms[:], rms[:])
                qn = qpool.tile([128, B], f32, name=f"qn{h}")
                nc.vector.tensor_mul(qn[:], q_ps[:], rms[:])
                # gamma * SCALE (per-partition scalar)
                nc.vector.tensor_scalar_mul(qn[:], qn[:], csg_sb[:, 1:2])
                # RoPE on partition halves: cos/sin stacked in csg col 0
                qr = qpool.tile([128, B], kv_dt, name=f"qr{h}")
                t1 = qpool.tile([64, B], f32, name=f"t1{h}")
                t2 = qpool.tile([64, B], f32, name=f"t2{h}")
                cos_ap = csg_sb[0:64, 0:1]
                sin_ap = csg_sb[64:128, 0:1]
                q1 = qn[0:64, :]
                q2 = qn[64:128, :]
                nc.vector.tensor_scalar_mul(t1[:], q1, cos_ap)
                nc.vector.tensor_scalar_mul(t2[:], q2, sin_ap)
                nc.vector.tensor_sub(qr[0:64, :], t1[:], t2[:])
                nc.vector.tensor_scalar_mul(t1[:], q2, cos_ap)
                nc.vector.tensor_scalar_mul(t2[:], q1, sin_ap)
                nc.vector.tensor_add(qr[64:128, :], t1[:], t2[:])
                q_heads.append(qr)

        # attention output columns, (128d, B) per head
        at_tiles = [qpool.tile([128, B], kv_dt, name=f"at{h}") for h in range(HC)]
        o_sb = qpool.tile([B, HID], f32, name="osb")

        # ---- main streamed attention loop (h-major; AV pipelined 1 unit
        # behind scores so the PE never waits on the ACT/DVE softmax chain) --
        units = [(h, b) for h in range(HC) for b in range(B)]
        # prefetch the LAST unit's k/v first so the tail isn't DMA-bound
        lastpool = ctx.enter_context(tc.tile_pool(name="lastkv", bufs=1))
        hl, bl = units[-1]
        k_last = lastpool.tile([128, S], kv_dt, name="klast")
        nc.sync.dma_start(k_last[:], kT[bl, hl])
        v_last = lastpool.tile([128, NCH, 128], kv_dt, name="vlast")
        nc.scalar.dma_start(v_last[:], vT[bl, hl])

        ps_sc = ctx.enter_context(tc.tile_pool(name="psS", bufs=2, space="PSUM"))
        ps_av = ctx.enter_context(tc.tile_pool(name="psV", bufs=2, space="PSUM"))
        ps_tot = ctx.enter_context(tc.tile_pool(name="psT", bufs=2, space="PSUM"))
        ps_o = ctx.enter_context(tc.tile_pool(name="psO", bufs=2, space="PSUM"))

        def emit_av(pend):
            v_p, probs_p, rowsum_p, h_p, b_p = pend
            av_ps = ps_av.tile([128, 1], f32, name="avps")
            for c in range(NCH):
                nc.tensor.matmul(
                    av_ps[:],
                    v_p[:, c, :],
                    probs_p[:, c:c + 1],
                    start=(c == 0), stop=(c == NCH - 1),
                )
            tot_ps = ps_tot.tile([128, 1], f32, name="totps")
            nc.tensor.matmul(tot_ps[:], ones_sb[:], rowsum_p[:],
                             start=True, stop=True)
            inv = spool.tile([128, 1], f32, name="inv")
            nc.vector.reciprocal(inv[:], tot_ps[:])
            nc.scalar.activation(at_tiles[h_p][:, b_p:b_p + 1], av_ps[:],
                                 mybir.ActivationFunctionType.Copy,
                                 scale=inv[:])

        def emit_oproj(h, first):
            # partial o_proj for one head into o_sb
            for n in range(HID // 512):
                o_ps = ps_o.tile([B, 512], f32, name="ops")
                nc.tensor.matmul(o_ps[:], at_tiles[h][:],
                                 wo_sb[:, h, n * 512:(n + 1) * 512],
                                 start=True, stop=True)
                if first:
                    nc.vector.tensor_copy(o_sb[:, n * 512:(n + 1) * 512], o_ps[:])
                else:
                    nc.vector.tensor_add(o_sb[:, n * 512:(n + 1) * 512],
                                         o_sb[:, n * 512:(n + 1) * 512], o_ps[:])

        pending = None
        for u, (h, b) in enumerate(units):
            last = (u == len(units) - 1)
            if last:
                k_sb, v_sb = k_last, v_last
            else:
                k_sb = kpool.tile([128, S], kv_dt, name="ktile")
                nc.sync.dma_start(k_sb[:], kT[b, h])
                v_sb = vpool.tile([128, NCH, 128], kv_dt, name="vtile")
                nc.scalar.dma_start(v_sb[:], vT[b, h])

            q_col = q_heads[h][:, b:b + 1]
            sc_ps = ps_sc.tile([128, NCH], f32, name="scps")
            for c in range(NCH):
                nc.tensor.matmul(
                    sc_ps[:, c:c + 1],
                    k_sb[:, c * 128:(c + 1) * 128],
                    q_col,
                    start=(c == 0), stop=(c == NCH - 1),
                )
            if pending is not None:
                emit_av(pending)
            sc_sb = spool.tile([128, NCH], f32, name="scsb")
            nc.vector.tensor_add(sc_sb[:], sc_ps[:], mask_tiles[b][:])
            probs = spool.tile([128, NCH], kv_dt, name="probs")
            rowsum = spool.tile([128, 1], f32, name="rowsum")
            nc.scalar.activation(probs[:], sc_sb[:],
                                 mybir.ActivationFunctionType.Exp,
                                 accum_out=rowsum[:])
            pending = (v_sb, probs, rowsum, h, b)
            if u == B + 1:
                # head 0's attention columns are complete; project mid-loop
                emit_oproj(0, first=True)
        emit_av(pending)
        emit_oproj(1, first=False)
        nc.sync.dma_start(out[:], o_sb[:])

    _split_sync_waits(nc)
    return nc


def _get_nc():
    if "nc" not in _CACHE:
        _CACHE["nc"] = _build_nc()
    return _CACHE["nc"]


def _shard_inputs(x, kv_k, kv_v, cos, sin, mask, Wq, Wo, q_gamma, kv_dtype=KV_DTYPE):
    if kv_dtype == "bf16":
        import ml_dtypes
        kv_np = ml_dtypes.bfloat16
    else:
        kv_np = np.float32

    x = np.ascontiguousarray(np.asarray(x, np.float32).reshape(B, HID))
    eye = np.eye(B, dtype=np.float32)
    csg = np.empty((D, 2), np.float32)
    csg[:64, 0] = np.asarray(cos, np.float32).reshape(-1)
    csg[64:, 0] = np.asarray(sin, np.float32).reshape(-1)
    csg[:, 1] = np.asarray(q_gamma, np.float32).reshape(-1) * SCALE
    mk = np.ascontiguousarray(
        np.asarray(mask, np.float32).reshape(B, NCH, 128).transpose(0, 2, 1))
    kv_k = np.asarray(kv_k, np.float32)
    kv_v = np.asarray(kv_v, np.float32)
    Wq = np.asarray(Wq, np.float32)
    Wo = np.asarray(Wo, np.float32)

    in_maps = []
    for c in range(NCORES):
        hs = c * HC
        # kT[b,hl] = kv_k[b,hs+hl]^T  (D, S) contiguous
        kt = np.ascontiguousarray(
            kv_k[:, hs:hs + HC].transpose(0, 1, 3, 2).astype(kv_np))
        # vT[b,hl][p, c*128+d] = kv_v[b,hs+hl][d, c*128+p]  (pre-tiled)
        vt = np.ascontiguousarray(
            kv_v[:, hs:hs + HC].reshape(B, HC, D, NCH, 128)
            .transpose(0, 1, 4, 3, 2).reshape(B, HC, 128, S).astype(kv_np))
        in_maps.append({
            "x_in": x,
            "eye8": eye,
            "wq": np.ascontiguousarray(Wq[:, hs * D:(hs + HC) * D].astype(kv_np)),
            "wo": np.ascontiguousarray(Wo[hs * D:(hs + HC) * D, :].astype(kv_np)),
            "kT": kt,
            "vT": vt,
            "csg": csg,
            "mk": mk,
        })
    return in_maps


def kernel(x, kv_k, kv_v, cos, sin, mask, Wq, Wo, q_gamma, _trace=False):
    from concourse.bass_utils import run_bass_kernel_spmd

    nc = _get_nc()
    in_maps = _shard_inputs(x, kv_k, kv_v, cos, sin, mask, Wq, Wo, q_gamma)
    res = run_bass_kernel_spmd(nc, in_maps, list(range(NCORES)), trace=_trace)
    acc = np.zeros((B, HID), np.float64)
    for c in range(NCORES):
        acc += res.results[c]["out"].astype(np.float64)
    out = acc.astype(np.float32).reshape(B, 1, HID)
    if _trace:
        return out, res
    return out


# revision 18
# speedup vs baseline: 1.0806x; 1.0085x over previous
"""Trainium2 Bass kernel for DrafterAttention (decode attention, B=8 H=16 D=128 S=4096 HID=2048).

Strategy (tensor-parallel over heads, 8 cores x 2 heads):
  - Host: shard Wq columns / Wo rows / kv on the head axis; pre-transpose
    kv_k -> (B,HC,D,S) and pre-tile kv_v -> (B,HC,128,NCH*128) so every
    device-side DMA moves contiguous per-partition lines and every matmul
    consumes natural SBUF layouts; pre-transpose x -> (HID,B).
  - Device (per core): qT = Wq_shard^T @ x^T on the PE; RMS-norm + RoPE in a
    (d-on-partition, batch-on-free) layout; per (b,h): 32 matmuls
    kT_chunk^T @ q_col -> scores (128s x 32chunk) in one PSUM accumulation
    group; exp via ACT with fused row-sum (no max subtraction: logits are O(1)
    by construction); partition-sum broadcast via ones-matmul; 32 accumulating
    matmuls vT_chunk^T @ prob_col -> attention head column; o_proj partial
    (8 x 2048) per core.
  - Host: sum the 8 partial outputs (the all-reduce).

KV/attention matmuls run in KV_DTYPE ("bf16" halves HBM traffic and runs the
PE weight path at FWL speed; "f32" is the exact fallback). The q/o projections
always run in f32.
"""
import numpy as np

B, H, D, S, HID = 8, 16, 128, 4096, 2048
NCORES = 8
HC = H // NCORES          # 2 heads per core
NCH = S // 128            # 32 s-chunks
SCALE = 1.0 / np.sqrt(D)
EPS = 1e-6

KV_DTYPE = "bf16"         # "bf16" | "f32"

_CACHE = {}


def _split_sync_waits(nc, max_waits=1):
    """This walrus build caps per-instruction sem waits; hoist any excess
    onto NoOp instructions inserted just before, on the same engine."""
    from concourse import mybir
    import bass_rust

    n = 0
    for fn in nc.m.functions:
        for blk in fn.blocks:
            new_list = []
            changed = False
            for inst in blk.instructions:
                si = inst.sync_info
                waits = list(si.on_wait) if (si and si.on_wait) else []
                if len(waits) > max_waits:
                    extra, keep = waits[:-max_waits], waits[-max_waits:]
                    for i in range(0, len(extra), max_waits):
                        n += 1
                        nop = bass_rust.InstNoOp(
                            name=f"I-waitsplit-{n}", ins=[], outs=[])
                        nop.engine = inst.engine
                        nop.sync_info = mybir.SyncInfo(
                            on_wait=extra[i:i + max_waits], on_update=[])
                        new_list.append(nop)
                    si.on_wait = keep
                    changed = True
                new_list.append(inst)
            if changed:
                blk.instructions[:] = new_list
    return n


def _build_nc(kv_dtype=KV_DTYPE):
    from contextlib import ExitStack
    import concourse.bass as bass
    import concourse.tile as tile
    from concourse import mybir

    f32 = mybir.dt.float32
    kv_dt = mybir.dt.bfloat16 if kv_dtype == "bf16" else f32

    nc = bass.Bass(trn_type="TRN2")

    x_in = nc.dram_tensor("x_in", [B, HID], f32, kind="ExternalInput")
    eye8 = nc.dram_tensor("eye8", [B, B], f32, kind="ExternalInput")
    wq = nc.dram_tensor("wq", [HID, HC * D], kv_dt, kind="ExternalInput")
    wo = nc.dram_tensor("wo", [HC * D, HID], kv_dt, kind="ExternalInput")
    # kv packed per (b,h): [kT (128,S) | vT pre-tiled (128,S)], 16KB lines
    kv = nc.dram_tensor("kv", [B, HC, 128, 2 * S], kv_dt, kind="ExternalInput")
    # csg: col0 = [cos;sin] stacked (128), col1 = gamma*SCALE (128)
    csg = nc.dram_tensor("csg", [D, 2], f32, kind="ExternalInput")
    mk = nc.dram_tensor("mk", [B, 128, NCH], f32, kind="ExternalInput")
    out = nc.dram_tensor("out", [B, HID], f32, kind="ExternalOutput")

    with ExitStack() as ctx:
        tc = ctx.enter_context(tile.TileContext(nc))

        consts = ctx.enter_context(tc.tile_pool(name="consts", bufs=1))
        qpool = ctx.enter_context(tc.tile_pool(name="qpool", bufs=1))
        kvpool = ctx.enter_context(tc.tile_pool(name="kvpool", bufs=6))
        spool = ctx.enter_context(tc.tile_pool(name="spool", bufs=2))

        # ---- constants / small inputs ----
        ones_sb = consts.tile([128, 128], f32)
        nc.gpsimd.memset(ones_sb[:], 1.0)
        eps_sb = consts.tile([128, 1], f32)
        nc.gpsimd.memset(eps_sb[:], EPS)
        csg_sb = consts.tile([D, 2], f32)
        nc.sync.dma_start(csg_sb[:], csg[:])
        x_sb = consts.tile([B, HID], f32)
        nc.sync.dma_start(x_sb[:], x_in[:])
        eye_sb = consts.tile([B, B], f32)
        nc.sync.dma_start(eye_sb[:], eye8[:])
        wq_sb = consts.tile([128, HID // 128, HC * D], kv_dt)
        nc.scalar.dma_start(wq_sb[:], wq[:].rearrange("(i p) j -> p i j", p=128))
        wo_sb = consts.tile([128, HC, HID], kv_dt)
        nc.scalar.dma_start(wo_sb[:], wo[:].rearrange("(h p) n -> p h n", p=128))
        # transpose x on the PE: 16 tiles (B,128) -> (128,B)
        xT_sb = consts.tile([128, HID // 128, B], kv_dt)
        with tc.tile_pool(name="psX", bufs=4, space="PSUM") as psx:
            for i in range(HID // 128):
                xt_ps = psx.tile([128, B], f32, name="xtps")
                nc.tensor.transpose(xt_ps[:], x_sb[:, i * 128:(i + 1) * 128],
                                    eye_sb[:])
                nc.scalar.copy(xT_sb[:, i, :], xt_ps[:])
        mask_tiles = []
        for b in range(B):
            mt = consts.tile([128, NCH], f32, name=f"mask{b}")
            nc.gpsimd.dma_start(mt[:], mk[b])
            mask_tiles.append(mt)

        # ---- q projection: qT_h = (Wq_h)^T @ x^T  -> (128d, B) per head ----
        q_heads = []
        with tc.tile_pool(name="psQ", bufs=1, space="PSUM") as psq:
            for h in range(HC):
                q_ps = psq.tile([128, B], f32, name=f"qps{h}")
                for i in range(HID // 128):
                    nc.tensor.matmul(
                        q_ps[:],
                        wq_sb[:, i, h * D:(h + 1) * D],
                        xT_sb[:, i, :],
                        start=(i == 0), stop=(i == HID // 128 - 1),
                    )
                # RMS norm (over the partition axis d) via ones-matmul
                sq = qpool.tile([128, B], f32, name=f"sq{h}")
                nc.scalar.square(sq[:], q_ps[:])
                ssq_ps = psq.tile([128, B], f32, name=f"ssq{h}")
                nc.tensor.matmul(ssq_ps[:], ones_sb[:], sq[:], start=True, stop=True)
                rms = qpool.tile([128, B], f32, name=f"rms{h}")
                nc.scalar.activation(rms[:], ssq_ps[:],
                                     mybir.ActivationFunctionType.Sqrt,
                                     bias=eps_sb[:], scale=1.0 / D)
                nc.vector.reciprocal(rms[:], rms[:])
                qn = qpool.tile([128, B], f32, name=f"qn{h}")
                nc.vector.tensor_mul(qn[:], q_ps[:], rms[:])
                # gamma * SCALE (per-partition scalar)
                nc.vector.tensor_scalar_mul(qn[:], qn[:], csg_sb[:, 1:2])
                # RoPE on partition halves: cos/sin stacked in csg col 0
                qr = qpool.tile([128, B], kv_dt, name=f"qr{h}")
                t1 = qpool.tile([64, B], f32, name=f"t1{h}")
                t2 = qpool.tile([64, B], f32, name=f"t2{h}")
                cos_ap = csg_sb[0:64, 0:1]
                sin_ap = csg_sb[64:128, 0:1]
                q1 = qn[0:64, :]
                q2 = qn[64:128, :]
                nc.vector.tensor_scalar_mul(t1[:], q1, cos_ap)
                nc.vector.tensor_scalar_mul(t2[:], q2, sin_ap)
                nc.vector.tensor_sub(qr[0:64, :], t1[:], t2[:])
                nc.vector.tensor_scalar_mul(t1[:], q2, cos_ap)
                nc.vector.tensor_scalar_mul(t2[:], q1, sin_ap)
                nc.vector.tensor_add(qr[64:128, :], t1[:], t2[:])
                q_heads.append(qr)

        # attention output columns, (128d, B) per head
        at_tiles = [qpool.tile([128, B], kv_dt, name=f"at{h}") for h in range(HC)]
        o_sb = qpool.tile([B, HID], f32, name="osb")

        # ---- main streamed attention loop (h-major; AV pipelined 1 unit
        # behind scores so the PE never waits on the ACT/DVE softmax chain) --
        units = [(h, b) for h in range(HC) for b in range(B)]
        # the LAST unit's kv is prefetched early (at u==4) so the tail
        # isn't DMA-bound
        lastpool = ctx.enter_context(tc.tile_pool(name="lastkv", bufs=1))
        hl, bl = units[-1]
        kv_last = lastpool.tile([128, 2 * S], kv_dt, name="kvlast")

        ps_sc = ctx.enter_context(tc.tile_pool(name="psS", bufs=2, space="PSUM"))
        ps_av = ctx.enter_context(tc.tile_pool(name="psV", bufs=2, space="PSUM"))
        ps_tot = ctx.enter_context(tc.tile_pool(name="psT", bufs=2, space="PSUM"))
        ps_o = ctx.enter_context(tc.tile_pool(name="psO", bufs=2, space="PSUM"))

        def emit_av(pend):
            kv_p, probs_p, rowsum_p, h_p, b_p = pend
            av_ps = ps_av.tile([128, 1], f32, name="avps")
            for c in range(NCH):
                nc.tensor.matmul(
                    av_ps[:],
                    kv_p[:, S + c * 128:S + (c + 1) * 128],
                    probs_p[:, c:c + 1],
                    start=(c == 0), stop=(c == NCH - 1),
                )
            tot_ps = ps_tot.tile([128, 1], f32, name="totps")
            nc.tensor.matmul(tot_ps[:], ones_sb[:], rowsum_p[:],
                             start=True, stop=True)
            inv = spool.tile([128, 1], f32, name="inv")
            nc.vector.reciprocal(inv[:], tot_ps[:])
            nc.scalar.activation(at_tiles[h_p][:, b_p:b_p + 1], av_ps[:],
                                 mybir.ActivationFunctionType.Copy,
                                 scale=inv[:])

        def emit_oproj(h, first):
            # partial o_proj for one head into o_sb; final head streams out
            for n in range(HID // 512):
                o_ps = ps_o.tile([B, 512], f32, name="ops")
                nc.tensor.matmul(o_ps[:], at_tiles[h][:],
                                 wo_sb[:, h, n * 512:(n + 1) * 512],
                                 start=True, stop=True)
                if first:
                    nc.vector.tensor_copy(o_sb[:, n * 512:(n + 1) * 512], o_ps[:])
                else:
                    nc.vector.tensor_add(o_sb[:, n * 512:(n + 1) * 512],
                                         o_sb[:, n * 512:(n + 1) * 512], o_ps[:])
                    nc.sync.dma_start(out[:, n * 512:(n + 1) * 512],
                                      o_sb[:, n * 512:(n + 1) * 512])

        pending = None
        rings = [nc.sync, nc.scalar]
        for u, (h, b) in enumerate(units):
            last = (u == len(units) - 1)
            if u == 4:
                nc.sync.dma_start(kv_last[:], kv[bl, hl])
            if last:
                kv_sb = kv_last
            else:
                kv_sb = kvpool.tile([128, 2 * S], kv_dt, name="kvtile")
                rings[u % 2].dma_start(kv_sb[:], kv[b, h])

            q_col = q_heads[h][:, b:b + 1]
            sc_ps = ps_sc.tile([128, NCH], f32, name="scps")
            for c in range(NCH):
                nc.tensor.matmul(
                    sc_ps[:, c:c + 1],
                    kv_sb[:, c * 128:(c + 1) * 128],
                    q_col,
                    start=(c == 0), stop=(c == NCH - 1),
                )
            if pending is not None:
                emit_av(pending)
            sc_sb = spool.tile([128, NCH], f32, name="scsb")
            nc.vector.tensor_add(sc_sb[:], sc_ps[:], mask_tiles[b][:])
            probs = spool.tile([128, NCH], kv_dt, name="probs")
            rowsum = spool.tile([128, 1], f32, name="rowsum")
            nc.scalar.activation(probs[:], sc_sb[:],
                                 mybir.ActivationFunctionType.Exp,
                                 accum_out=rowsum[:])
            pending = (kv_sb, probs, rowsum, h, b)
            if u == B + 1:
                # head 0's attention columns are complete; project mid-loop
                emit_oproj(0, first=True)
        emit_av(pending)
        emit_oproj(1, first=False)

    _split_sync_waits(nc)
    return nc


def _get_nc():
    if "nc" not in _CACHE:
        _CACHE["nc"] = _build_nc()
    return _CACHE["nc"]


def _shard_inputs(x, kv_k, kv_v, cos, sin, mask, Wq, Wo, q_gamma, kv_dtype=KV_DTYPE):
    if kv_dtype == "bf16":
        import ml_dtypes
        kv_np = ml_dtypes.bfloat16
    else:
        kv_np = np.float32

    x = np.ascontiguousarray(np.asarray(x, np.float32).reshape(B, HID))
    eye = np.eye(B, dtype=np.float32)
    csg = np.empty((D, 2), np.float32)
    csg[:64, 0] = np.asarray(cos, np.float32).reshape(-1)
    csg[64:, 0] = np.asarray(sin, np.float32).reshape(-1)
    csg[:, 1] = np.asarray(q_gamma, np.float32).reshape(-1) * SCALE
    mk = np.ascontiguousarray(
        np.asarray(mask, np.float32).reshape(B, NCH, 128).transpose(0, 2, 1))
    kv_k = np.asarray(kv_k, np.float32)
    kv_v = np.asarray(kv_v, np.float32)
    Wq = np.asarray(Wq, np.float32)
    Wo = np.asarray(Wo, np.float32)

    in_maps = []
    for c in range(NCORES):
        hs = c * HC
        # packed per (b,h): [kT (128,S) | vT pre-tiled (128,S)]
        kvp = np.empty((B, HC, 128, 2 * S), kv_np)
        kvp[..., :S] = kv_k[:, hs:hs + HC].transpose(0, 1, 3, 2)
        kvp[..., S:] = (kv_v[:, hs:hs + HC].reshape(B, HC, D, NCH, 128)
                        .transpose(0, 1, 4, 3, 2).reshape(B, HC, 128, S))
        in_maps.append({
            "x_in": x,
            "eye8": eye,
            "wq": np.ascontiguousarray(Wq[:, hs * D:(hs + HC) * D].astype(kv_np)),
            "wo": np.ascontiguousarray(Wo[hs * D:(hs + HC) * D, :].astype(kv_np)),
            "kv": kvp,
            "csg": csg,
            "mk": mk,
        })
    return in_maps


def kernel(x, kv_k, kv_v, cos, sin, mask, Wq, Wo, q_gamma, _trace=False):
    from concourse.bass_utils import run_bass_kernel_spmd

    nc = _get_nc()
    in_maps = _shard_inputs(x, kv_k, kv_v, cos, sin, mask, Wq, Wo, q_gamma)
    res = run_bass_kernel_spmd(nc, in_maps, list(range(NCORES)), trace=_trace)
    acc = np.zeros((B, HID), np.float64)
    for c in range(NCORES):
        acc += res.results[c]["out"].astype(np.float64)
    out = acc.astype(np.float32).reshape(B, 1, HID)
    if _trace:
        return out, res
    return out
